# revision 4
# baseline (speedup 1.0000x reference)
"""Trainium2 Bass kernel for quantized 1x1-conv + BatchNorm(train) + MultiStepLIF.

Strategy (8 NeuronCores, data-parallel over batch B=16 -> 2 per core):

  Phase A1 (hi):  y_hi[t,b,o,hw] = sum_c bf16(x*scale_q)[..] * w_int[o,c]
                  (PE bf16, 1024-wide moving operand), evicted to SBUF f32
                  with free row-sums on ACT; sum-of-squares on DVE.
                  BN stats are taken from the HI pass only: the bf16
                  rounding error averages down to ~1e-5 over 65k samples.
  AllReduce      of the 2x[128,2] stats right after A1 (~1/3 into the
                  kernel) so its ~35us latency overlaps phase A2.
  Phase A2 (lo):  y += y_lo via PE matmuls + DVE add-evict, fully hidden
                  under the collective window.
  Phase B (LIF):  u' = a*y + (b-1) on ACT;  carry recurrence on DVE via a
                  fused custom op  c = sel(u'+c < 0, (u'+c)*0.5 + 0.5, 0);
                  spike indicator w = bf16(u' + c_prev) on GpSimd (sign of
                  w == v-1 is exact in bf16).  Host decodes spike = (w>=0).

Precision: x*scale_q split hi/lo into two bf16 streams (w_int exact in
bf16), accumulated in fp32 PSUM -> ~1e-5 relative error on y. LIF/BN
elementwise work is fp32; only the output indicator is bf16.
"""

import sys

for _p in ("/opt/trn_rl_repo",):
    if _p not in sys.path:
        sys.path.insert(0, _p)

import numpy as np
import ml_dtypes

import concourse.bass as bass
import concourse.mybir as mybir
import concourse.tile as tile
from concourse import bacc
from concourse import bass_utils
from concourse import dve_ops as _dve_ops
from concourse.dve_spec import (
    C0,
    C1,
    C2,
    Spec as _DveSpec,
    Src0,
    Src1,
    Zero,
    select as _dve_select,
)

BF16 = ml_dtypes.bfloat16
F32 = mybir.dt.float32
BF = mybir.dt.bfloat16
ALU = mybir.AluOpType
ACTF = mybir.ActivationFunctionType

T, B, C, H, W = 4, 16, 256, 32, 32
O = 256
NCORES = 8
BC = B // NCORES          # batches per core
TBC = T * BC              # 8 (t,b) pairs per core
HWP = H * W               # 1024
NTOT = float(T * B * H * W)  # positions per channel, global
EPS = 1e-5
N_WARM_MM = 26            # dummy matmuls to lift the PE HAM clock gate


def _register_lif_w_op():
    """Custom fused DVE op: out = (src0+src1) < s0 ? (src0+src1)*s1 + imm2 : 0.

    With state w = v - 1 (v the LIF membrane), u' = a*y + (b-1):
        w_t = u'_t + c_{t-1},   c_t = [w_t < 0] * (0.5*w_t + 0.5)
    This op computes c_t from (u'_t, c_{t-1}) in one pass
    (s0=0, s1=0.5, imm2=0.5). Spike_t <=> w_t >= 0."""
    name = "LIF_STEP_W_ANT"
    for op in _dve_ops.OPS:
        if op.name == name:
            return op
    v = Src0 + Src1
    spec = _DveSpec(
        body=_dve_select(v < C0, v * C1 + C2, Zero),
        reference=lambda in0, in1, s0, s1, imm2: np.where(
            (in0.astype(np.float32) + in1) < s0,
            (in0.astype(np.float32) + in1) * s1 + imm2,
            0.0,
        ).astype(np.float32),
    )
    op = _dve_ops.DveOp(name, spec, subdim=False, uops_sha={"v3": "4799c5c6b3c029f2"})
    _dve_ops.OPS.append(op)
    _dve_ops.CUSTOM_DVE_SPECS[name] = spec
    _dve_ops._SUB_OPCODE_FOR_NAME[name] = (
        _dve_ops._CUSTOM_DVE_ROW_BASE + len(_dve_ops.OPS) - 1
    )
    return op


_LIF_OP = _register_lif_w_op()


def _build_nc():
    nc = bacc.Bacc(
        "TRN2",
        target_bir_lowering=False,
        debug=False,
        num_devices=NCORES,
    )
    xhi = nc.dram_tensor("xhi", [TBC, 128, 2, HWP], BF, kind="ExternalInput")
    xlo = nc.dram_tensor("xlo", [TBC, 128, 2, HWP], BF, kind="ExternalInput")
    wT = nc.dram_tensor("wT", [128, 2, O], BF, kind="ExternalInput")
    gb = nc.dram_tensor("gb", [128, 4], F32, kind="ExternalInput")
    out = nc.dram_tensor("m0", [T, 2, 128, BC * HWP], BF, kind="ExternalOutput")

    xhi_ap = xhi.ap()
    xlo_ap = xlo.ap()
    out_ap = out.ap()

    with tile.TileContext(nc) as tc:
        with (
            tc.tile_pool(name="consts", bufs=1) as consts,
            tc.tile_pool(name="xhpool", bufs=3) as xhpool,
            tc.tile_pool(name="xlpool", bufs=3) as xlpool,
            tc.tile_pool(name="ypool", bufs=1) as ypool,
            tc.tile_pool(name="pspool", bufs=3, space="PSUM") as pspool,
            tc.tile_pool(name="warmps", bufs=1, space="PSUM") as warmps,
            tc.tile_pool(name="scrpool", bufs=2) as scrpool,
            tc.tile_pool(name="small", bufs=1) as small,
            tc.tile_pool(name="dram", bufs=1, space="DRAM") as dram,
            tc.tile_pool(name="upool", bufs=3) as upool,
            tc.tile_pool(name="wpool", bufs=3) as wpool,
            tc.tile_pool(name="cpool", bufs=4) as cpool,
        ):
            # ---- constants; first x tile gets the front of the DMA queue ----
            xh0 = xhpool.tile([128, 2, HWP], BF, name="x_h", tag="x_h")
            nc.sync.dma_start(xh0[:], xhi_ap[0])
            w_sb = consts.tile([128, 2, O], BF, name="w_sb")
            nc.sync.dma_start(w_sb[:], wT.ap())
            gb_sb = consts.tile([128, 4], F32, name="gb_sb")
            nc.sync.dma_start(gb_sb[:], gb.ap())

            # zero tiles: junk feeds the PE warm-up, zc is the LIF carry seed
            junk = consts.tile([128, 128], BF, name="junk")
            nc.vector.memset(junk[:], 0.0)
            zc = consts.tile([128, BC * HWP], F32, name="zc")
            nc.vector.memset(zc[:], 0.0)

            # Preload the sqrt ACT table so the post-collective sqrt is cheap.
            junkf = small.tile([128, 1], F32, name="junkf")
            nc.scalar.activation(junkf[:], gb_sb[:, 0:1], ACTF.Sqrt)

            # ---- PE warm-up: ~3us of dummy matmuls lifts HAM to full clock
            wps = warmps.tile([128, 128], F32, name="wps")
            for i in range(N_WARM_MM):
                nc.tensor.matmul(
                    wps[:],
                    lhsT=junk[:],
                    rhs=junk[:],
                    start=True,
                    stop=True,
                    skip_group_check=True,
                )

            # ---- phase A1: hi matmuls + stats ----
            y_sb = [
                ypool.tile([128, TBC * HWP], F32, name=f"ysb{ot}") for ot in range(2)
            ]
            sums = small.tile([128, 2 * TBC], F32, name="sums")
            ssqs = small.tile([128, 2 * TBC], F32, name="ssqs")

            for tb in range(TBC):
                if tb == 0:
                    x_t = xh0
                else:
                    x_t = xhpool.tile([128, 2, HWP], BF, name="x_h", tag="x_h")
                    nc.sync.dma_start(x_t[:], xhi_ap[tb])
                for ot in range(2):
                    ps = pspool.tile([128, HWP], F32, name="ps", tag="ps")
                    for ch in range(2):
                        for chunk in range(2):
                            nc.tensor.matmul(
                                ps[:, chunk * 512 : (chunk + 1) * 512],
                                lhsT=w_sb[:, ch, ot * 128 : (ot + 1) * 128],
                                rhs=x_t[:, ch, chunk * 512 : (chunk + 1) * 512],
                                start=(ch == 0),
                                stop=(ch == 1),
                                skip_group_check=True,
                            )
                    ysl = y_sb[ot][:, tb * HWP : (tb + 1) * HWP]
                    col = 2 * tb + ot
                    # evict PSUM -> SBUF + free row-sum on ACT
                    nc.scalar.activation(
                        ysl,
                        ps[:],
                        ACTF.Copy,
                        bias=0.0,
                        scale=1.0,
                        accum_out=sums[:, col : col + 1],
                    )
                    # sum of squares in one DVE pass (output discarded)
                    scr = scrpool.tile([128, HWP], F32, name="scr", tag="scr")
                    nc.vector.scalar_tensor_tensor(
                        out=scr[:],
                        in0=ysl,
                        scalar=0.0,
                        in1=ysl,
                        op0=ALU.bypass,
                        op1=ALU.mult,
                        accum_out=ssqs[:, col : col + 1],
                    )

            # ---- finalize local stats, AllReduce ----
            stats4 = small.tile([128, 4], F32, name="stats4")
            nc.vector.tensor_reduce(
                stats4[:, 0:2],
                sums[:].rearrange("p (t o) -> p o t", o=2),
                axis=mybir.AxisListType.X,
                op=ALU.add,
            )
            nc.vector.tensor_reduce(
                stats4[:, 2:4],
                ssqs[:].rearrange("p (t o) -> p o t", o=2),
                axis=mybir.AxisListType.X,
                op=ALU.add,
            )

            cc_in = dram.tile([128, 4], F32, name="cc_in")
            cc_out = dram.tile([128, 4], F32, name="cc_out")
            # cc_in upload from ACT's hwdge queue (sync is busy with x tiles)
            nc.scalar.dma_start(cc_in[:], stats4[:])
            nc.gpsimd.collective_compute(
                "AllReduce",
                ALU.add,
                replica_groups=[list(range(NCORES))],
                ins=[cc_in.opt()],
                outs=[cc_out.opt()],
            )
            gstat = small.tile([128, 4], F32, name="gstat")
            nc.scalar.dma_start(gstat[:], cc_out[:])

            # ---- phase A2: lo matmuls, added in place (under the collective)
            for tb in range(TBC):
                xl = xlpool.tile([128, 2, HWP], BF, name="x_l", tag="x_l")
                nc.sync.dma_start(xl[:], xlo_ap[tb])
                for ot in range(2):
                    ps = pspool.tile([128, HWP], F32, name="ps", tag="ps")
                    for ch in range(2):
                        for chunk in range(2):
                            nc.tensor.matmul(
                                ps[:, chunk * 512 : (chunk + 1) * 512],
                                lhsT=w_sb[:, ch, ot * 128 : (ot + 1) * 128],
                                rhs=xl[:, ch, chunk * 512 : (chunk + 1) * 512],
                                start=(ch == 0),
                                stop=(ch == 1),
                                skip_group_check=True,
                            )
                    ysl = y_sb[ot][:, tb * HWP : (tb + 1) * HWP]
                    nc.vector.tensor_tensor(ysl, ysl, ps[:], ALU.add)

            # ---- small math: a = 0.5*inv, b = 0.5*(beta - mean*inv) - 1 ----
            # bh0 = 0.5*beta - 1 is collective-independent; compute it early.
            bh0 = small.tile([128, 2], F32, name="bh0")
            t3 = small.tile([128, 2], F32, name="t3")
            nc.vector.tensor_scalar(t3[:], gb_sb[:, 2:4], 0.5, None, ALU.mult)
            nc.vector.tensor_scalar(bh0[:], t3[:], -1.0, None, ALU.add)

            aab = small.tile([128, 4], F32, name="aab")
            mean = small.tile([128, 2], F32, name="mean")
            e2 = small.tile([128, 2], F32, name="e2")
            msq = small.tile([128, 2], F32, name="msq")
            vare = small.tile([128, 2], F32, name="vare")
            sq = small.tile([128, 2], F32, name="sq")
            dd = small.tile([128, 2], F32, name="dd")
            t1 = small.tile([128, 2], F32, name="t1")
            inv = small.tile([128, 2], F32, name="inv")
            mi = small.tile([128, 2], F32, name="mi")
            rs = small.tile([128, 2], F32, name="rs")

            inv_n = 1.0 / NTOT
            nc.vector.tensor_scalar(mean[:], gstat[:, 0:2], inv_n, None, ALU.mult)
            nc.vector.tensor_scalar(e2[:], gstat[:, 2:4], inv_n, None, ALU.mult)
            nc.vector.tensor_tensor(msq[:], mean[:], mean[:], ALU.mult)
            nc.vector.tensor_tensor(t1[:], e2[:], msq[:], ALU.subtract)
            nc.vector.tensor_scalar(vare[:], t1[:], EPS, None, ALU.add)
            nc.scalar.activation(sq[:], vare[:], ACTF.Sqrt)
            # one Newton refinement: s = 0.5*(s + vare/s)
            nc.vector.reciprocal(rs[:], sq[:])
            nc.vector.tensor_tensor(dd[:], vare[:], rs[:], ALU.mult)
            nc.vector.tensor_tensor(t1[:], sq[:], dd[:], ALU.add)
            nc.vector.tensor_scalar(sq[:], t1[:], 0.5, None, ALU.mult)
            # inv = gamma / sqrt(var+eps)
            nc.vector.reciprocal(rs[:], sq[:])
            nc.vector.tensor_tensor(inv[:], gb_sb[:, 0:2], rs[:], ALU.mult)
            nc.vector.tensor_scalar(aab[:, 0:2], inv[:], 0.5, None, ALU.mult)
            nc.vector.tensor_tensor(mi[:], mean[:], inv[:], ALU.mult)
            nc.vector.scalar_tensor_tensor(
                out=aab[:, 2:4],
                in0=mi[:],
                scalar=-0.5,
                in1=bh0[:],
                op0=ALU.mult,
                op1=ALU.add,
            )

            # ---- phase B: LIF over T (state w = v-1; carry c) ----
            carry = [zc, zc]
            for t in range(T):
                for ot in range(2):
                    ysl = y_sb[ot][:, t * BC * HWP : (t + 1) * BC * HWP]
                    u = upool.tile([128, BC * HWP], F32, name="u", tag="u")
                    nc.scalar.activation(
                        u[:],
                        ysl,
                        ACTF.Identity,
                        bias=aab[:, 2 + ot : 3 + ot],
                        scale=aab[:, ot : ot + 1],
                    )
                    # spike indicator w_t = bf16(u' + c_prev) on GpSimd;
                    # runs concurrently with the DVE carry update below.
                    wv = wpool.tile([128, BC * HWP], BF, name="wv", tag="wv")
                    nc.gpsimd.tensor_tensor(wv[:], u[:], carry[ot][:], ALU.add)
                    nc.sync.dma_start(out_ap[t, ot], wv[:])
                    if t < T - 1:
                        cnew = cpool.tile([128, BC * HWP], F32, name="carry", tag="carry")
                        nc.vector._custom_dve(
                            _LIF_OP,
                            out=cnew[:],
                            in0=u[:],
                            in1=carry[ot][:],
                            s0=0.0,
                            s1=0.5,
                            imm2=0.5,
                        )
                        carry[ot] = cnew

    nc.compile()
    return nc


_NC_CACHE = None


def _get_nc():
    global _NC_CACHE
    if _NC_CACHE is None:
        _NC_CACHE = _build_nc()
    return _NC_CACHE


def _prep_inputs(x, w, gamma, beta):
    x = np.ascontiguousarray(np.asarray(x, dtype=np.float32))
    w = np.asarray(w, dtype=np.float32)
    gamma = np.asarray(gamma, dtype=np.float32)
    beta = np.asarray(beta, dtype=np.float32)

    # fake-quant weights exactly like the reference forward pass
    scale = (np.max(np.abs(w)) / np.float32(127.0)).astype(np.float32)
    wint = np.clip(np.rint((w / scale).astype(np.float32)), -127.0, 127.0).astype(
        np.float32
    )
    # lhsT layout: [cc(128), ch(2), O]  (w_int values are exact in bf16)
    wT_packed = np.ascontiguousarray(
        wint.T.reshape(2, 128, O).transpose(1, 0, 2)
    ).astype(BF16)

    gb_packed = np.zeros((128, 4), np.float32)
    gb_packed[:, 0] = gamma[:128]
    gb_packed[:, 1] = gamma[128:]
    gb_packed[:, 2] = beta[:128]
    gb_packed[:, 3] = beta[128:]

    # scale_q folded into x so PSUM accumulates in real units
    xs = (x.reshape(T, NCORES, BC, C, HWP) * scale).astype(np.float32)
    in_maps = []
    for c in range(NCORES):
        xf = np.ascontiguousarray(xs[:, c]).reshape(T * BC, 2, 128, HWP)
        hi = xf.astype(BF16)
        lo = (xf - hi.astype(np.float32)).astype(BF16)
        # [tb, ch, cc, hw] -> [tb, cc, ch, hw]
        hi = np.ascontiguousarray(hi.transpose(0, 2, 1, 3))
        lo = np.ascontiguousarray(lo.transpose(0, 2, 1, 3))
        in_maps.append({"xhi": hi, "xlo": lo, "wT": wT_packed, "gb": gb_packed})
    return in_maps


def _assemble(results):
    spikes = np.empty((T, B, O, H, W), np.float32)
    for c in range(NCORES):
        m = results[c]["m0"]  # [T, 2, 128, BC*HWP] bf16 w-values; w>=0 <=> spike
        s = (m >= 0).astype(np.float32)
        sm = s.reshape(T, 2, 128, BC, HWP).transpose(0, 3, 1, 2, 4)  # [t,b,ot,oc,hw]
        spikes[:, c * BC : (c + 1) * BC] = sm.reshape(T, BC, O, H, W)
    return spikes


def run(x, w, gamma, beta, trace=False, **spmd_kwargs):
    in_maps = _prep_inputs(x, w, gamma, beta)
    nc = _get_nc()
    res = bass_utils.run_bass_kernel_spmd(
        nc, in_maps, core_ids=list(range(NCORES)), trace=trace, **spmd_kwargs
    )
    return _assemble(res.results), res


def kernel(x, w, gamma, beta):
    spikes, _ = run(x, w, gamma, beta)
    return spikes
